# revision 19
# baseline (speedup 1.0000x reference)
# Trainium2 Bass kernel for CrossScaleFreqAttention.
#
# Math (per batch b):
#   tokens[l, n, c] = mean over the 8x8 window of {target, 4 neighbors}[l, c]
#   proj = tokens @ proj_w + proj_b ; q/k/v linear ; softmax over n (5)
#   delta[l, c] = (attn-weighted v) @ out_w + out_b
#   out = target_win + delta broadcast over the window
#
# Sharding: data-parallel over B=8 -> one batch element per NeuronCore,
# weights replicated, no cross-core communication.
#
# Per-core structure (memory-bound: 80 MiB in + 16 MiB out per core;
# the 16 SDMA engines sustain ~390 GB/s when busy => ~260 us of DMA
# work is the floor, so the schedule aims for zero DMA idle):
#   L=1024 in 8 chunks of 128 SBUF partitions, emitted SOFTWARE-
#   PIPELINED in two stages per chunk:
#     A(i): loads + window pooling + token transpose
#     B(i): attention chain + broadcast-add + store
#   as A0 A1 B0 A2 B1 ... A7 B6 B7. Every engine's in-order queue then
#   holds chunk i+1's early ops BEFORE chunk i's late ops, so at the
#   tail chunk 7's chain overlaps chunk 6's instead of queuing behind
#   it (the chains share the scalar/vector queues).
#   - Queues: neighbor loads (f32 -> bf16 in-DMA) on the gpsimd SWDGE
#     ring; target loads + early stores + chunk 7's store on the sync
#     HWDGE queue (orders: T0 T1 st0 T2 st1 T3 st2 T4..T7 st7 — each
#     store's add is ready ~20 us before the following load must
#     issue); weights on the scalar HWDGE queue.
#   - Stores for chunks 3-6 are issued at the END of the SWDGE ring,
#     behind every neighbor load, so the DMA engines drain 8 MiB of
#     held stores exactly while chunk 7's compute chain runs.
#   - Neighbor pooling on the TensorEngine per neighbor k (8
#     accumulating matmuls with a stationary bf16 identity, one
#     16B-cacheline w-octet innermost), so chunk 7's pooling trails
#     only the last neighbor's DMA, not all four. Target pooling on
#     the VectorE (its tile must stay f32 for the exact in-place add).
#   - Token/attention chain in bf16 (fp32 PSUM accumulate everywhere;
#     delta is ~0.1% of the output, so bf16 rounding is ~1e-6 there).

import math
import os

import numpy as np

B, L, C, W2 = 8, 1024, 64, 64
K, NTOK, D = 4, 5, 32
LCHUNK = 128
NCHUNK = L // LCHUNK
HALF = 64  # l-positions per half-chunk (320 = HALF*NTOK columns <= 512 PSUM)
NCORES = 8
DEFER = (2, 3, 4, 5, 6)  # chunks whose stores are held to the end of the run

LAST_RESULTS = None  # BassKernelResults of the most recent run (for test.py)


def _build():
    from contextlib import ExitStack

    import concourse.bacc as bacc
    import concourse.mybir as mybir
    import concourse.tile as tile

    f32 = mybir.dt.float32
    bf16 = mybir.dt.bfloat16
    AX = mybir.AxisListType.X
    EXP = mybir.ActivationFunctionType.Exp
    MULT = mybir.AluOpType.mult
    ADD = mybir.AluOpType.add

    nc = bacc.Bacc(
        "TRN2",
        target_bir_lowering=False,
        debug=False,
        num_devices=NCORES,
    )

    def din(name, shape, dt=f32):
        return nc.dram_tensor(name, shape, dt, kind="ExternalInput").ap()

    tgt = din("tgt", [L, C * W2])
    nbr = din("nbr", [K, L, C * W2])
    ident = din("ident", [128, 128], bf16)
    pw = din("pw", [C, D], bf16)  # pre-scaled by 1/64 (window mean) on host
    pb = din("pb", [D])
    qw = din("qw", [D, D], bf16)  # pre-scaled by 1/sqrt(D) on host
    qb = din("qb", [D])           # pre-scaled by 1/sqrt(D) on host
    kw = din("kw", [D, D], bf16)
    kb = din("kb", [D])
    vw = din("vw", [D, D], bf16)
    vb = din("vb", [D])
    # out_w augmented with out_b as row D and a unit column C: the
    # attention output is kept UNNORMALIZED (weighted by raw exp
    # scores) with the softmax denominator carried as row D of fused,
    # so ow_aug^T @ fused_aug = [delta*den + ob*den | den] and one
    # per-partition multiply by 1/den after the transpose yields
    # delta + ob exactly.
    ow = din("ow", [D + 1, C + 1], bf16)
    y = nc.dram_tensor("y", [L, C * W2], f32, kind="ExternalOutput").ap()

    with (
        tile.TileContext(nc) as tc,
        ExitStack() as ctx,
        nc.allow_low_precision(reason="bf16 attention path; output add stays f32"),
    ):
        const = ctx.enter_context(tc.tile_pool(name="const", bufs=1))
        targp = ctx.enter_context(tc.tile_pool(name="targ", bufs=7))
        nbrp = ctx.enter_context(tc.tile_pool(name="nbr", bufs=2))
        tokp = ctx.enter_context(tc.tile_pool(name="tok", bufs=2))
        smallp = ctx.enter_context(tc.tile_pool(name="small", bufs=2))
        ps_tok = ctx.enter_context(tc.tile_pool(name="ps_tok", bufs=1, space="PSUM"))
        ps_tt = ctx.enter_context(tc.tile_pool(name="ps_tt", bufs=1, space="PSUM"))
        ps_sm = ctx.enter_context(tc.tile_pool(name="ps_sm", bufs=3, space="PSUM"))

        # Weights ride the scalar HWDGE queue: the load/store queues stay
        # untouched so the first big load issues immediately.
        ident_s = const.tile([128, 128], bf16)
        nc.scalar.dma_start(out=ident_s[:], in_=ident)
        pw_s = const.tile([C, D], bf16)
        nc.scalar.dma_start(out=pw_s[:], in_=pw)
        qw_s = const.tile([D, D], bf16)
        nc.scalar.dma_start(out=qw_s[:], in_=qw)
        kw_s = const.tile([D, D], bf16)
        nc.scalar.dma_start(out=kw_s[:], in_=kw)
        vw_s = const.tile([D, D], bf16)
        nc.scalar.dma_start(out=vw_s[:], in_=vw)
        ow_s = const.tile([D + 1, C], bf16)
        nc.scalar.dma_start(out=ow_s[:], in_=ow)
        pb_s = const.tile([D, 1], f32)
        nc.scalar.dma_start(out=pb_s[:], in_=pb.unsqueeze(1))
        qb_s = const.tile([D, 1], f32)
        nc.scalar.dma_start(out=qb_s[:], in_=qb.unsqueeze(1))
        kb_s = const.tile([D, 1], f32)
        nc.scalar.dma_start(out=kb_s[:], in_=kb.unsqueeze(1))
        vb_s = const.tile([D, 1], f32)
        nc.scalar.dma_start(out=vb_s[:], in_=vb.unsqueeze(1))
        ones_d = const.tile([D, 1], bf16)
        nc.vector.memset(ones_d[:], 1.0)
        ones_1 = const.tile([1, D], bf16)
        nc.vector.memset(ones_1[:], 1.0)

        state = {}  # chunk -> (targ, tokT) carried from stage A to B

        def emit_store(i, h, engine):
            cs = slice(h * (C // 2), (h + 1) * (C // 2))
            yv = y[i * LCHUNK : (i + 1) * LCHUNK].rearrange(
                "l (c w) -> l c w", w=W2
            )
            engine.dma_start(out=yv[:, cs], in_=state[i][0][:, cs])

        def emit_A(i):
            l0 = i * LCHUNK

            # ---- loads: target f32 on the sync HWDGE queue, neighbors
            # (f32 -> bf16 in-DMA) on the SWDGE FIFO ring.
            targ = targp.tile([LCHUNK, C, W2], f32, tag="targ")
            nc.sync.dma_start(
                out=targ[:],
                in_=tgt[l0 : l0 + LCHUNK].rearrange("l (c w) -> l c w", w=W2),
            )
            nbig = nbrp.tile([LCHUNK, K, C, W2], bf16, tag="nbig")
            for k in range(K):
                nc.gpsimd.dma_start(
                    out=nbig[:, k],
                    in_=nbr[k, l0 : l0 + LCHUNK].rearrange("l (c w) -> l c w", w=W2),
                )

            # ---- window pooling ----
            tok_t = tokp.tile([LCHUNK, C], bf16, tag="tok_t")
            tok_n = tokp.tile([LCHUNK, K * C], bf16, tag="tok_n")
            ptok8 = ps_tok.tile([LCHUNK, K, 4, 16, 8], f32, tag="ptok")
            nc.vector.reduce_sum(tok_t[:], targ[:], axis=AX)
            nbig_v = nbig.rearrange("l k (cg c) w -> l k cg c w", cg=4)
            for k in range(K):
                for wo in range(8):
                    nc.tensor.matmul(
                        ptok8[:, k],
                        lhsT=ident_s[:],
                        rhs=nbig_v[:, k, :, :, 8 * wo : 8 * (wo + 1)],
                        start=(wo == 0),
                        stop=(wo == 7),
                    )
                nc.vector.reduce_sum(
                    tok_n[:, k * C : (k + 1) * C].rearrange(
                        "l (cg c) -> l cg c", cg=4
                    ),
                    ptok8[:, k],
                    axis=AX,
                )

            # ---- transpose tokens to [c, (l,n)] (l-major columns) ----
            tokT = tokp.tile([C, LCHUNK * NTOK], bf16, tag="tokT")
            tokT_ln = tokT.rearrange("c (l n) -> c l n", n=NTOK)
            for n in range(NTOK):
                ttp = ps_tt.tile([C, LCHUNK], bf16, tag="ttp")
                src_n = tok_t[:] if n == 0 else tok_n[:, (n - 1) * C : n * C]
                nc.tensor.transpose(ttp[:], src_n, ident_s[:])
                nc.scalar.copy(tokT_ln[:, :, n], ttp[:])

            state[i] = (targ, tokT)

        def emit_B(i):
            targ, tokT = state[i]

            # row D carries the softmax denominator (see ow_aug note)
            fusedT = smallp.tile([D + 1, LCHUNK], bf16, tag="fusedT")
            exps = smallp.tile([1, LCHUNK * NTOK], bf16, tag="exps")
            projs2 = []

            for h in range(2):
                cols = slice(h * HALF * NTOK, (h + 1) * HALF * NTOK)

                # proj = tokens @ pw + pb   -> [D, 320] (d on partitions)
                pproj = ps_sm.tile([D, HALF * NTOK], f32, tag="sm")
                nc.tensor.matmul(pproj[:], lhsT=pw_s[:], rhs=tokT[:, cols])
                projs = smallp.tile([D, HALF * NTOK], bf16, tag="projs")
                nc.scalar.add(projs[:], pproj[:], pb_s[:])

                # k / v over all tokens, q over token 0 only
                pk = ps_sm.tile([D, HALF * NTOK], f32, tag="sm")
                nc.tensor.matmul(pk[:], lhsT=kw_s[:], rhs=projs[:])
                ks = smallp.tile([D, HALF * NTOK], bf16, tag="ks")
                nc.scalar.add(ks[:], pk[:], kb_s[:])

                pv = ps_sm.tile([D, HALF * NTOK], f32, tag="sm")
                nc.tensor.matmul(pv[:], lhsT=vw_s[:], rhs=projs[:])
                vs = smallp.tile([D, HALF * NTOK], bf16, tag="vs")
                nc.scalar.add(vs[:], pv[:], vb_s[:])

                pq = ps_sm.tile([D, HALF], f32, tag="sm")
                nc.tensor.matmul(
                    pq[:],
                    lhsT=qw_s[:],
                    rhs=projs.rearrange("d (l n) -> d l n", n=NTOK)[:, :, 0],
                )
                qs = smallp.tile([D, HALF], bf16, tag="qs")
                nc.scalar.add(qs[:], pq[:], qb_s[:])

                # scores[l, n] = sum_d q[d, l] * k[d, (l,n)]
                qk = smallp.tile([D, HALF * NTOK], bf16, tag="qk")
                nc.vector.tensor_mul(
                    qk.rearrange("d (l n) -> d l n", n=NTOK),
                    ks.rearrange("d (l n) -> d l n", n=NTOK),
                    qs.unsqueeze(2).to_broadcast([D, HALF, NTOK]),
                )
                psc = ps_sm.tile([1, HALF * NTOK], f32, tag="sm")
                nc.tensor.matmul(psc[:], lhsT=ones_d[:], rhs=qk[:])
                # scores are O(1e-2): exp without max-shift is exact enough
                nc.scalar.activation(exps[:, cols], psc[:], EXP)
                projs2.append(vs)

            # softmax denominator -> row D of fused (no normalization of
            # the weights; 1/den is applied per-l after the transpose)
            nc.vector.reduce_sum(
                fusedT[D : D + 1, :],
                exps.rearrange("p (l n) -> p l n", n=NTOK),
                axis=AX,
            )

            for h in range(2):
                cols = slice(h * HALF * NTOK, (h + 1) * HALF * NTOK)
                # broadcast raw exp weights over d, weight v, reduce over n
                pab = ps_sm.tile([D, HALF * NTOK], f32, tag="sm")
                nc.tensor.matmul(pab[:], lhsT=ones_1[:], rhs=exps[:, cols])
                av = smallp.tile([D, HALF * NTOK], bf16, tag="av")
                nc.vector.tensor_mul(av[:], projs2[h][:], pab[:])
                nc.vector.reduce_sum(
                    fusedT[:D, h * HALF : (h + 1) * HALF],
                    av.rearrange("d (l n) -> d l n", n=NTOK),
                    axis=AX,
                )

            # delta*den (+ ob*den in the same matmul) -> [c, l], with den
            # appended as row C, transposed together to [l, c | den]
            pdelta = ps_sm.tile([C, LCHUNK], f32, tag="sm")
            nc.tensor.matmul(pdelta[:], lhsT=ow_s[:], rhs=fusedT[:])
            deltaT = smallp.tile([C + 1, LCHUNK], bf16, tag="deltaT")
            nc.scalar.copy(deltaT[:C], pdelta[:])
            nc.scalar.copy(deltaT[C : C + 1], fusedT[D : D + 1, :])
            pdT = ps_sm.tile([LCHUNK, C + 1], bf16, tag="sm")
            nc.tensor.transpose(pdT[:], deltaT[:], ident_s[: C + 1, : C + 1])
            rdenT = smallp.tile([LCHUNK, 1], f32, tag="rdenT")
            nc.vector.reciprocal(rdenT[:], pdT[:, C : C + 1])

            # in-place targ += pdT * (1/den), broadcast over the window,
            # on the VectorE; store halves pipeline against the adds
            # (held chunks store later)
            for h in range(2):
                cs = slice(h * (C // 2), (h + 1) * (C // 2))
                nc.vector.scalar_tensor_tensor(
                    targ[:, cs],
                    pdT[:, cs].unsqueeze(2).to_broadcast([LCHUNK, C // 2, W2]),
                    rdenT[:],
                    targ[:, cs],
                    op0=MULT,
                    op1=ADD,
                )
                if i not in DEFER:
                    emit_store(i, h, nc.sync)

        emit_A(0)
        emit_A(1)
        for i in range(NCHUNK):
            emit_B(i)
            if i + 2 < NCHUNK:
                emit_A(i + 2)

        # Held stores: issued at the END of the SWDGE ring, after every
        # neighbor load, so the DMA engines finish all input traffic
        # first and drain these 8 MiB while chunk 7's compute chain
        # completes.
        for j in DEFER:
            emit_store(j, 0, nc.gpsimd)
            emit_store(j, 1, nc.gpsimd)

    nc.compile()
    return nc


def kernel(
    target_win,
    neighbor_wins,
    proj_w,
    proj_b,
    q_w,
    q_b,
    k_w,
    k_b,
    v_w,
    v_b,
    out_w,
    out_b,
):
    global LAST_RESULTS
    import ml_dtypes

    from concourse.bass_utils import run_bass_kernel_spmd

    f = np.float32
    bf = ml_dtypes.bfloat16
    target_win = np.ascontiguousarray(np.asarray(target_win, f))
    neighbor_wins = np.ascontiguousarray(np.asarray(neighbor_wins, f))
    # Fold the window-mean (1/64) into proj_w and the 1/sqrt(D) score
    # scale into q_w/q_b (linear ops commute with these scalings).
    pw = (np.asarray(proj_w, f) / float(W2)).astype(bf)
    sc = 1.0 / math.sqrt(D)
    qw = (np.asarray(q_w, f) * sc).astype(bf)
    qb = np.asarray(q_b, f) * sc
    shared = {
        "ident": np.eye(128, dtype=bf),
        "pw": pw,
        "pb": np.asarray(proj_b, f),
        "qw": qw,
        "qb": qb,
        "kw": np.asarray(k_w, f).astype(bf),
        "kb": np.asarray(k_b, f),
        "vw": np.asarray(v_w, f).astype(bf),
        "vb": np.asarray(v_b, f),
        "ow": np.vstack(
            [np.asarray(out_w, f), np.asarray(out_b, f)[None, :]]
        ).astype(bf),
    }
    in_maps = []
    for b in range(NCORES):
        in_maps.append(
            {
                "tgt": target_win[b].reshape(L, C * W2),
                "nbr": np.ascontiguousarray(
                    neighbor_wins[:, b].reshape(K, L, C * W2)
                ),
                **shared,
            }
        )

    nc = _build()
    res = run_bass_kernel_spmd(
        nc,
        in_maps,
        list(range(NCORES)),
        trace=bool(os.environ.get("KERNEL_PROFILE")),
    )
    LAST_RESULTS = res
    out = np.stack(
        [res.results[b]["y"].reshape(L, C, 8, 8) for b in range(NCORES)]
    )
    return out.astype(np.float32, copy=False)
